# revision 6
# baseline (speedup 1.0000x reference)
"""Trainium2 Bass kernel for nn_ConditioningGNN.

Math (see reference): ctx = H[context_ids]; one-query MHA of path_token over
ctx -> upd; upd = LN1(upd + path_token); attn2 = W_out2 @ (W_v2 @ upd) (+biases);
out = LN2(attn2 + H)  broadcast over all N nodes.

Distribution: H sharded row-wise over 8 NeuronCores; path token / context rows /
weights replicated. Each core redundantly computes the tiny attention prelude
on-device (PE in bf16 + DVE/ACT in f32), then streams its H shard through a
DMA-bound broadcast-add + LayerNorm loop.

Main-loop layout (the kernel is HBM-bound: 25.6 MB in + 25.6 MB out per core
vs ~358 GB/s per-NC HBM): blocks of 128*KB consecutive rows in p-major order
(partition p holds KB contiguous DRAM rows), so each block moves as ONE
dma_start with 128 descriptors of KB*2KB contiguous bytes — big descriptors,
few transfers. The input DMA is a gpsimd (SWDGE) cast-DMA f32->bf16: HBM-side
bytes are unchanged but the SBUF tile halves, and every DVE/GpSimd
element-wise op runs at 2x 16-bit throughput — without this DVE (adds +
bn_stats) is the steady-state bottleneck at ~96% busy. Input DMAs are
issued PF blocks ahead so gpsimd's in-order stream never stalls on its own
transfer. Per block: DVE/GpSimd split the broadcast adds (bf16), DVE
bn_stats/bn_aggr per 128-row slice, ACT normalizes bf16 -> f32 output tile,
one dma_start out on the otherwise-idle SP queue. bf16 H costs ~4e-3 rel
err on the normalized output, well inside the 2e-2 gate.

Host side does only data movement: shard H, gather the 64 context rows,
pre-transpose weights into the K-major layout PE needs, bf16-cast the tiny
replicated prelude operands.
"""

import numpy as np
import ml_dtypes
from contextlib import ExitStack

import concourse.bass as bass
import concourse.tile as tile
from concourse import bacc, mybir
from concourse.bass_utils import run_bass_kernel_spmd
from concourse.masks import make_identity

N = 100000
D = 512
C = 64
NH = 8
DH = D // NH
P = 128
R = D // P  # 4 column/row chunks of 128
N_CORES = 8
NSH = N // N_CORES
LN_EPS = 1e-5
KB = 8    # rows per partition per main-loop block (p-major); block = 128*KB rows
XBUFS = 8   # bf16 input tiles in flight
OBUFS = 4   # f32 output tiles in flight
PF = 5      # input-DMA prefetch distance (blocks)

F32 = mybir.dt.float32
BF16 = mybir.dt.bfloat16
AF = mybir.ActivationFunctionType
OP = mybir.AluOpType

_cache: dict = {}

_W_NAMES = ("w1oT", "wv2T", "w2oT")
# bf16 row vectors (PE rank-1 operands); ln2 rows stay f32
_BROW_NAMES = ("pt", "b1o", "bv2", "b2o", "ln1g", "ln1b")
_FROW_NAMES = ("ln2g", "ln2b")


def _build(nsh: int, kb: int, general: bool, zb: bool, ln1_triv: bool):
    nc = bacc.Bacc("TRN2", target_bir_lowering=False, debug=False)

    h = nc.dram_tensor("h", [nsh, D], F32, kind="ExternalInput")
    o_dram = nc.dram_tensor("out", [nsh, D], F32, kind="ExternalOutput")
    pc_d = nc.dram_tensor("pcT", [D, 1 + C], BF16, kind="ExternalInput")
    wqkv_d = nc.dram_tensor("wqkvT", [D, 3 * D], BF16, kind="ExternalInput")
    bqkv_d = nc.dram_tensor("bqkv", [3 * D], BF16, kind="ExternalInput")
    w_d = {nm: nc.dram_tensor(nm, [D, D], BF16, kind="ExternalInput")
           for nm in _W_NAMES}
    row_d = {nm: nc.dram_tensor(nm, [D], BF16, kind="ExternalInput")
             for nm in _BROW_NAMES}
    frow_d = {nm: nc.dram_tensor(nm, [D], F32, kind="ExternalInput")
              for nm in _FROW_NAMES}

    with tile.TileContext(nc) as tc, ExitStack() as ctx:
        singles = ctx.enter_context(tc.tile_pool(name="singles", bufs=1))
        ppool = ctx.enter_context(tc.tile_pool(name="ppool", bufs=3, space="PSUM"))
        xpool = ctx.enter_context(tc.tile_pool(name="xpool", bufs=XBUFS))
        opool = ctx.enter_context(tc.tile_pool(name="opool", bufs=OBUFS))
        spool = ctx.enter_context(tc.tile_pool(name="spool", bufs=4))

        # ---------- constants / small loads (before weights: tiny) ----------
        ident = singles.tile([P, P], F32, tag="ident")
        make_identity(nc, ident)
        ones_f = singles.tile([1, P], F32, tag="ones_f")
        nc.vector.memset(ones_f, 1.0)
        ones_b = singles.tile([1, P], BF16, tag="ones_b")
        nc.vector.memset(ones_b, 1.0)
        one1 = ones_b[0:1, 0:1]
        eps_col = singles.tile([P, 1], F32, tag="eps_col")
        nc.vector.memset(eps_col, LN_EPS)

        pc_sb = singles.tile([P, R, 1 + C], BF16, tag="pc_sb")
        nc.sync.dma_start(pc_sb, pc_d[:].rearrange("(r p) c -> p r c", p=P))
        bqkv_sb = singles.tile([1, 3 * D], BF16, tag="bqkv_sb")
        nc.sync.dma_start(bqkv_sb, bqkv_d[:].rearrange("(a d) -> a d", a=1))
        rows = {}
        for nm in _BROW_NAMES:
            t = singles.tile([1, D], BF16, tag="row_" + nm)
            nc.sync.dma_start(t, row_d[nm][:].rearrange("(a d) -> a d", a=1))
            rows[nm] = t
        frows = {}
        for nm in _FROW_NAMES:
            t = singles.tile([1, D], F32, tag="row_" + nm)
            nc.sync.dma_start(t, frow_d[nm][:].rearrange("(a d) -> a d", a=1))
            frows[nm] = t
        wqkv_sb = singles.tile([P, R, 3 * D], BF16, tag="wqkvT")
        nc.sync.dma_start(wqkv_sb,
                          wqkv_d[:].rearrange("(r p) n -> p r n", p=P))
        w_sb = {}
        for nm in _W_NAMES:
            t = singles.tile([P, R, D], BF16, tag=nm)
            nc.sync.dma_start(t, w_d[nm][:].rearrange("(r p) n -> p r n", p=P))
            w_sb[nm] = t

        # vector-matrix product: psum[1, D] = sum_r colT[:, r] . wT[:, r, :]
        # plus rank-1 (ones) adds of bf16 rows, all accumulated on PE.
        def vec_mat(psum, col_sb, w_t, add_rows):
            last = R - 1 if not add_rows else None
            for r in range(R):
                nc.tensor.matmul(psum, lhsT=col_sb[:, r:r + 1], rhs=w_t[:, r, :],
                                 start=(r == 0), stop=(r == last))
            for j, br in enumerate(add_rows):
                nc.tensor.matmul(psum, lhsT=one1, rhs=br,
                                 start=False, stop=(j == len(add_rows) - 1))

        # bf16 row [1, D] -> column chunks [P, R] via outer product with 1
        def row_to_col(row_sb, tag):
            cps = ppool.tile([P, R], F32, tag="ps")
            for r in range(R):
                nc.tensor.matmul(cps[:, r:r + 1],
                                 lhsT=row_sb[0:1, r * P:(r + 1) * P],
                                 rhs=one1, start=True, stop=True)
            csb = singles.tile([P, R], BF16, tag=tag)
            nc.scalar.copy(csb, cps)
            return csb

        # ---------- prelude: path attention -> attn2 ----------
        # one fused projection: [kh; vh; q](+junk) = [ctx|pt].T @ [Wq|Wk|Wv].T
        # kv_ps rows 0..C-1 = kh (j=1 slice), vh (j=2); row C = q (j=0;
        # partition C=64 keeps the engine partition-offset aligned).
        # Rank-1 bias adds hit junk rows/cols harmlessly.
        kv_ps = ppool.tile([1 + C, 3 * D], F32, tag="ps_big", bufs=1)
        for j in range(3):
            for r in range(R):
                nc.tensor.matmul(kv_ps[:, j * D:(j + 1) * D],
                                 lhsT=pc_sb[:, r, :],
                                 rhs=wqkv_sb[:, r, j * D:(j + 1) * D],
                                 start=(r == 0), stop=(zb and r == R - 1))
            if not zb:
                nc.tensor.matmul(kv_ps[:, j * D:(j + 1) * D],
                                 lhsT=ones_b[0:1, 0:1 + C],
                                 rhs=bqkv_sb[0:1, j * D:(j + 1) * D],
                                 start=False, stop=True)
        q_row = singles.tile([1, D], BF16, tag="q_row")
        nc.scalar.copy(q_row, kv_ps[C:C + 1, 0:D])
        vh_sb = singles.tile([C, D], BF16, tag="vh_sb")
        nc.scalar.copy(vh_sb, kv_ps[0:C, 2 * D:3 * D])

        # scores^T [keys, heads] = sum_d (q (x) ones) * kh, per head
        qb_ps = ppool.tile([C, D], F32, tag="ps")
        nc.tensor.matmul(qb_ps, lhsT=ones_b[0:1, 0:C], rhs=q_row,
                         start=True, stop=True)
        qb_sb = singles.tile([C, D], F32, tag="qb_sb")
        nc.scalar.copy(qb_sb, qb_ps)
        s1 = singles.tile([C, D], F32, tag="s1")
        nc.vector.tensor_mul(s1, qb_sb, kv_ps[0:C, D:2 * D])
        sc_t = singles.tile([C, NH], F32, tag="sc_t")
        nc.vector.reduce_sum(sc_t, s1[:, :].rearrange("p (h d) -> p h d", h=NH),
                             axis=mybir.AxisListType.X)

        # transpose -> [heads, keys]; softmax over keys (scores are O(0.1):
        # exp without max-subtraction is safe at this weight scale)
        st_ps = ppool.tile([NH, C], F32, tag="ps")
        nc.tensor.transpose(st_ps, sc_t, ident[0:C, 0:C])
        wrow = singles.tile([NH, C], F32, tag="wrow")
        ssum = singles.tile([NH, 1], F32, tag="ssum")
        nc.scalar.activation(wrow, st_ps, AF.Exp, bias=0.0, scale=DH ** -0.5,
                             accum_out=ssum)
        rs = singles.tile([NH, 1], F32, tag="rs")
        nc.vector.reciprocal(rs, ssum)
        nc.vector.tensor_scalar_mul(wrow, wrow, rs)

        # w^T [keys, heads] for the per-head attention matmuls
        wT_ps = ppool.tile([C, NH], F32, tag="ps")
        nc.tensor.transpose(wT_ps, wrow, ident[0:NH, 0:NH])
        wT_sb = singles.tile([C, NH], BF16, tag="wT_sb")
        nc.scalar.copy(wT_sb, wT_ps)

        # attn1 (pre-out-proj) directly in column form [P, R]:
        # per head: [DH, 1] = vh_head^T . w_head
        a1c_ps = ppool.tile([P, R], F32, tag="ps")
        for hh in range(NH):
            po = DH * (hh % 2)
            nc.tensor.matmul(a1c_ps[po:po + DH, hh // 2:hh // 2 + 1],
                             lhsT=vh_sb[:, hh * DH:(hh + 1) * DH],
                             rhs=wT_sb[:, hh:hh + 1], start=True, stop=True)
        a1c_sb = singles.tile([P, R], BF16, tag="a1c_sb")
        nc.scalar.copy(a1c_sb, a1c_ps)

        # t = attn1 @ W_out1.T (+ b_out1) + pt    [1, D]
        t_ps = ppool.tile([1, D], F32, tag="ps")
        vec_mat(t_ps, a1c_sb, w_sb["w1oT"],
                (rows["pt"],) if zb else (rows["b1o"], rows["pt"]))

        # upd = LN1(t) (* ln1g + ln1b)
        st6 = singles.tile([1, 6], F32, tag="st6")
        nc.vector.bn_stats(st6, t_ps)
        mv1 = singles.tile([1, 2], F32, tag="mv1")
        nc.vector.bn_aggr(mv1, st6)
        sg1 = singles.tile([1, 1], F32, tag="sg1")
        nc.scalar.activation(sg1, mv1[0:1, 1:2], AF.Sqrt, bias=eps_col[0:1, :])
        rstd1 = singles.tile([1, 1], F32, tag="rstd1")
        nc.vector.reciprocal(rstd1, sg1)
        u_row = singles.tile([1, D], BF16, tag="u_row")
        nc.vector.tensor_scalar(u_row, t_ps, scalar1=mv1[0:1, 0:1],
                                scalar2=rstd1, op0=OP.subtract, op1=OP.mult)
        if not ln1_triv:
            nc.vector.tensor_mul(u_row, u_row, rows["ln1g"])
            nc.vector.tensor_add(u_row, u_row, rows["ln1b"])

        # v = upd @ Wv2.T (+ bv2);  attn2 = v @ W_out2.T (+ b_out2)
        uc_sb = row_to_col(u_row, "uc_sb")
        v_ps = ppool.tile([1, D], F32, tag="ps")
        vec_mat(v_ps, uc_sb, w_sb["wv2T"], () if zb else (rows["bv2"],))
        v_row = singles.tile([1, D], BF16, tag="v_row")
        nc.scalar.copy(v_row, v_ps)
        vc_sb = row_to_col(v_row, "vc_sb")
        a2_ps = ppool.tile([1, D], F32, tag="ps")
        vec_mat(a2_ps, vc_sb, w_sb["w2oT"], () if zb else (rows["b2o"],))
        a2_row = singles.tile([1, D], F32, tag="a2_row")
        nc.scalar.copy(a2_row, a2_ps)

        # ---------- main loop: out = LN2(attn2 + H) ----------
        # a_b is the attn2 row broadcast to all 128 partitions. Blocks of
        # 128*kb consecutive rows, p-major (partition p holds rows
        # [s+p*kb, s+(p+1)*kb)): one input dma_start per block with 16 KB
        # contiguous descriptors on the SP queue, one output dma_start per
        # block on the ACT queue. DVE/GpSimd split the broadcast adds, DVE
        # does stats, ACT normalizes in place. The loop is software-
        # pipelined one block deep so DVE keeps working while GpSimd's
        # slower adds finish.
        ab_ps = ppool.tile([P, D], F32, tag="ps")
        nc.tensor.matmul(ab_ps, lhsT=ones_f, rhs=a2_row, start=True, stop=True)
        a_b = singles.tile([P, D], F32, tag="a_b")
        nc.vector.tensor_copy(a_b, ab_ps)
        if general:
            def bcast_row(row_sb, tag):
                bps = ppool.tile([P, D], F32, tag="ps")
                nc.tensor.matmul(bps, lhsT=ones_f, rhs=row_sb,
                                 start=True, stop=True)
                bsb = singles.tile([P, D], F32, tag=tag)
                nc.vector.tensor_copy(bsb, bps)
                return bsb

            g2b = bcast_row(frows["ln2g"], "g2b")
            b2b = bcast_row(frows["ln2b"], "b2b")

        # a_b in bf16 for the 16-bit adds
        a_bh = singles.tile([P, D], BF16, tag="a_bh")
        nc.vector.tensor_copy(a_bh, a_b)

        # (start_row, partitions, rows-per-partition) per block
        blocks = []
        s = 0
        while s + P * kb <= nsh:
            blocks.append((s, P, kb))
            s += P * kb
        rem = nsh - s
        if rem:
            # tail: pick rows-per-partition so partitions stay <= P
            kt = (rem + P - 1) // P
            while rem % kt:
                kt += 1
            blocks.append((s, rem // kt, kt))

        def in_dma(blk):
            s, rp, kbb = blk
            x_t = xpool.tile([rp, kbb, D], BF16, tag="x")
            nc.gpsimd.dma_start(
                x_t, h[s:s + rp * kbb, :].rearrange("(p a) d -> p a d", p=rp))
            return x_t

        def adds(state):
            (s, rp, kbb), x_t = state
            for a in range(kbb):
                eng = nc.vector if a % 2 == 0 else nc.gpsimd
                eng.tensor_add(x_t[:, a, :], x_t[:, a, :], a_bh[:rp, :])

        def back(state):
            (s, rp, kbb), x_t = state
            st = spool.tile([rp, kbb, 6], F32, tag="st")
            mv = spool.tile([rp, kbb, 2], F32, tag="mv")
            sg = spool.tile([rp, kbb], F32, tag="sg")
            nmr = spool.tile([rp, kbb], F32, tag="nmr")
            o_t = opool.tile([rp, kbb, D], F32, tag="o")
            for a in range(kbb):
                nc.vector.bn_stats(st[:, a, :], x_t[:, a, :])
                nc.vector.bn_aggr(mv[:, a, :], st[:, a, :])
            nc.scalar.activation(sg, mv[:, :, 1], AF.Sqrt, bias=eps_col[:rp, :])
            nc.vector.reciprocal(sg, sg)
            nc.vector.tensor_mul(nmr, mv[:, :, 0], sg)
            nc.vector.tensor_scalar_mul(nmr, nmr, -1.0)
            for a in range(kbb):
                nc.scalar.activation(o_t[:, a, :], x_t[:, a, :], AF.Identity,
                                     bias=nmr[:, a:a + 1], scale=sg[:, a:a + 1])
                if general:
                    nc.vector.tensor_mul(o_t[:, a, :], o_t[:, a, :], g2b[:rp, :])
                    nc.vector.tensor_add(o_t[:, a, :], o_t[:, a, :], b2b[:rp, :])
            nc.sync.dma_start(
                o_dram[s:s + rp * kbb, :].rearrange("(p a) d -> p a d", p=rp),
                o_t)

        nb = len(blocks)
        states = {}
        for i in range(min(PF, nb)):
            states[i] = (blocks[i], in_dma(blocks[i]))
        prev = None
        for i in range(nb):
            if i + PF < nb:
                states[i + PF] = (blocks[i + PF], in_dma(blocks[i + PF]))
            adds(states[i])
            if prev is not None:
                back(states.pop(prev))
            prev = i
        back(states.pop(prev))

    nc.compile()
    return nc


def _get_nc(nsh, kb, general, zb, ln1_triv):
    key = (nsh, kb, general, zb, ln1_triv)
    if key not in _cache:
        _cache[key] = _build(nsh, kb, general, zb, ln1_triv)
    return _cache[key]


def _prep_in_maps(inputs: dict):
    f = lambda x: np.ascontiguousarray(np.asarray(x), dtype=np.float32)
    bf = lambda x: np.ascontiguousarray(np.asarray(x, dtype=np.float32),
                                        dtype=ml_dtypes.bfloat16)
    H = f(inputs["H"])
    assert H.shape == (N, D), H.shape
    cid = np.asarray(inputs["context_ids"]).astype(np.int64)
    W_in1 = f(inputs["W_in1"])
    b_in1 = f(inputs["b_in1"])
    W_in2 = f(inputs["W_in2"])
    b_in2 = f(inputs["b_in2"])
    ln1g, ln1b = f(inputs["ln1_g"]), f(inputs["ln1_b"])
    ln2g, ln2b = f(inputs["ln2_g"]), f(inputs["ln2_b"])
    biases = (b_in1, f(inputs["b_out1"]), b_in2[2 * D:], f(inputs["b_out2"]))
    zb = all(np.all(b == 0.0) for b in biases)
    ln1_triv = bool(np.all(ln1g == 1.0) and np.all(ln1b == 0.0))
    general = not (np.all(ln2g == 1.0) and np.all(ln2b == 0.0))
    pt32 = f(inputs["path_token"])
    common = {
        "pcT": bf(np.concatenate([H[cid].T, pt32[:, None]], axis=1)),
        "pt": bf(pt32),
        "wqkvT": bf(W_in1.T),
        "bqkv": bf(b_in1),
        "w1oT": bf(f(inputs["W_out1"]).T),
        "wv2T": bf(W_in2[2 * D:3 * D].T),
        "w2oT": bf(f(inputs["W_out2"]).T),
        "b1o": bf(inputs["b_out1"]),
        "bv2": bf(b_in2[2 * D:3 * D]),
        "b2o": bf(inputs["b_out2"]),
        "ln1g": bf(ln1g),
        "ln1b": bf(ln1b),
        "ln2g": ln2g,
        "ln2b": ln2b,
    }
    shards = np.split(H, N_CORES, axis=0)
    in_maps = [dict(common, h=shards[i]) for i in range(N_CORES)]
    return in_maps, (general, zb, ln1_triv)


def _run(inputs: dict, trace: bool = False):
    in_maps, (general, zb, ln1_triv) = _prep_in_maps(inputs)
    nc = _get_nc(NSH, KB, general, zb, ln1_triv)
    res = run_bass_kernel_spmd(nc, in_maps, core_ids=list(range(N_CORES)),
                               trace=trace)
    out = np.concatenate([res.results[i]["out"] for i in range(N_CORES)], axis=0)
    return out, res


def kernel(**inputs) -> np.ndarray:
    out, _ = _run(inputs)
    return out


# revision 12
# speedup vs baseline: 1.0242x; 1.0242x over previous
"""Trainium2 Bass kernel for nn_ConditioningGNN.

Math (see reference): ctx = H[context_ids]; one-query MHA of path_token over
ctx -> upd; upd = LN1(upd + path_token); attn2 = W_out2 @ (W_v2 @ upd) (+biases);
out = LN2(attn2 + H)  broadcast over all N nodes.

Distribution: H sharded row-wise over 8 NeuronCores; path token / context rows /
weights replicated. Each core redundantly computes the tiny attention prelude
on-device (PE in bf16 + DVE/ACT in f32), then streams its H shard through a
DMA-bound broadcast-add + LayerNorm loop.

Main-loop layout (the kernel is HBM-bound: 25.6 MB in + 25.6 MB out per core
vs ~358 GB/s per-NC HBM): blocks of 128*KB consecutive rows in p-major order
(partition p holds KB contiguous DRAM rows), so each block moves as ONE
dma_start with 128 descriptors of KB*2KB contiguous bytes — big descriptors,
few transfers. Input stream rides the SP HWDGE queue (SP's stream is nothing
but these DMAs, so it prefetches XBUFS deep on its own), output stream the
ACT HWDGE queue. Per block: DVE takes 3/8 of the broadcast adds and GpSimd
5/8 (GpSimd's adds are ~2x slower and DVE also carries all bn_stats — DVE
is the scarce engine), each from its OWN copy of the broadcast row to avoid
SBUF conflicts on a shared operand; DVE bn_stats/bn_aggr per 128-row slice;
ACT normalizes IN PLACE (the same SBUF buffer is DMA'd back out).

Host side does only data movement: shard H, gather the 64 context rows,
pre-transpose weights into the K-major layout PE needs, bf16-cast the tiny
replicated prelude operands.
"""

import numpy as np
import ml_dtypes
from contextlib import ExitStack

import concourse.bass as bass
import concourse.tile as tile
from concourse import bacc, mybir
from concourse.bass_utils import run_bass_kernel_spmd
from concourse.masks import make_identity

N = 100000
D = 512
C = 64
NH = 8
DH = D // NH
P = 128
R = D // P  # 4 column/row chunks of 128
N_CORES = 8
NSH = N // N_CORES
LN_EPS = 1e-5
KB = 8    # rows per partition per main-loop block (p-major); block = 128*KB rows
XBUFS = 9   # f32 tiles in flight (input, then normalized in place for output)

F32 = mybir.dt.float32
BF16 = mybir.dt.bfloat16
AF = mybir.ActivationFunctionType
OP = mybir.AluOpType

_cache: dict = {}

_W_NAMES = ("w1oT", "wv2T", "w2oT")
# bf16 row vectors (PE rank-1 operands); ln2 rows stay f32
_BROW_NAMES = ("pt", "b1o", "bv2", "b2o", "ln1g", "ln1b")
_FROW_NAMES = ("ln2g", "ln2b")


def _build(nsh: int, kb: int, general: bool, zb: bool, ln1_triv: bool):
    nc = bacc.Bacc("TRN2", target_bir_lowering=False, debug=False)

    h = nc.dram_tensor("h", [nsh, D], F32, kind="ExternalInput")
    o_dram = nc.dram_tensor("out", [nsh, D], F32, kind="ExternalOutput")
    pc_d = nc.dram_tensor("pcT", [D, 1 + C], BF16, kind="ExternalInput")
    wqkv_d = nc.dram_tensor("wqkvT", [D, 3 * D], BF16, kind="ExternalInput")
    bqkv_d = nc.dram_tensor("bqkv", [3 * D], BF16, kind="ExternalInput")
    w_d = {nm: nc.dram_tensor(nm, [D, D], BF16, kind="ExternalInput")
           for nm in _W_NAMES}
    row_d = {nm: nc.dram_tensor(nm, [D], BF16, kind="ExternalInput")
             for nm in _BROW_NAMES}
    frow_d = {nm: nc.dram_tensor(nm, [D], F32, kind="ExternalInput")
              for nm in _FROW_NAMES}

    with tile.TileContext(nc) as tc, ExitStack() as ctx:
        singles = ctx.enter_context(tc.tile_pool(name="singles", bufs=1))
        ppool = ctx.enter_context(tc.tile_pool(name="ppool", bufs=3, space="PSUM"))
        xpool = ctx.enter_context(tc.tile_pool(name="xpool", bufs=XBUFS))
        spool = ctx.enter_context(tc.tile_pool(name="spool", bufs=4))

        # ---------- constants / small loads (before weights: tiny) ----------
        ident = singles.tile([P, P], F32, tag="ident")
        make_identity(nc, ident)
        ones_f = singles.tile([1, P], F32, tag="ones_f")
        nc.vector.memset(ones_f, 1.0)
        ones_b = singles.tile([1, P], BF16, tag="ones_b")
        nc.vector.memset(ones_b, 1.0)
        one1 = ones_b[0:1, 0:1]
        eps_col = singles.tile([P, 1], F32, tag="eps_col")
        nc.vector.memset(eps_col, LN_EPS)

        pc_sb = singles.tile([P, R, 1 + C], BF16, tag="pc_sb")
        nc.sync.dma_start(pc_sb, pc_d[:].rearrange("(r p) c -> p r c", p=P))
        bqkv_sb = singles.tile([1, 3 * D], BF16, tag="bqkv_sb")
        nc.sync.dma_start(bqkv_sb, bqkv_d[:].rearrange("(a d) -> a d", a=1))
        rows = {}
        for nm in _BROW_NAMES:
            t = singles.tile([1, D], BF16, tag="row_" + nm)
            nc.sync.dma_start(t, row_d[nm][:].rearrange("(a d) -> a d", a=1))
            rows[nm] = t
        frows = {}
        for nm in _FROW_NAMES:
            t = singles.tile([1, D], F32, tag="row_" + nm)
            nc.sync.dma_start(t, frow_d[nm][:].rearrange("(a d) -> a d", a=1))
            frows[nm] = t
        wqkv_sb = singles.tile([P, R, 3 * D], BF16, tag="wqkvT")
        nc.sync.dma_start(wqkv_sb,
                          wqkv_d[:].rearrange("(r p) n -> p r n", p=P))
        w_sb = {}
        for nm in _W_NAMES:
            t = singles.tile([P, R, D], BF16, tag=nm)
            nc.sync.dma_start(t, w_d[nm][:].rearrange("(r p) n -> p r n", p=P))
            w_sb[nm] = t

        # vector-matrix product: psum[1, D] = sum_r colT[:, r] . wT[:, r, :]
        # plus rank-1 (ones) adds of bf16 rows, all accumulated on PE.
        def vec_mat(psum, col_sb, w_t, add_rows):
            last = R - 1 if not add_rows else None
            for r in range(R):
                nc.tensor.matmul(psum, lhsT=col_sb[:, r:r + 1], rhs=w_t[:, r, :],
                                 start=(r == 0), stop=(r == last))
            for j, br in enumerate(add_rows):
                nc.tensor.matmul(psum, lhsT=one1, rhs=br,
                                 start=False, stop=(j == len(add_rows) - 1))

        # bf16 row [1, D] -> column chunks [P, R] via outer product with 1
        def row_to_col(row_sb, tag):
            cps = ppool.tile([P, R], F32, tag="ps")
            for r in range(R):
                nc.tensor.matmul(cps[:, r:r + 1],
                                 lhsT=row_sb[0:1, r * P:(r + 1) * P],
                                 rhs=one1, start=True, stop=True)
            csb = singles.tile([P, R], BF16, tag=tag)
            nc.scalar.copy(csb, cps)
            return csb

        # ---------- prelude: path attention -> attn2 ----------
        # one fused projection: [kh; vh; q](+junk) = [ctx|pt].T @ [Wq|Wk|Wv].T
        # kv_ps rows 0..C-1 = kh (j=1 slice), vh (j=2); row C = q (j=0;
        # partition C=64 keeps the engine partition-offset aligned).
        # Rank-1 bias adds hit junk rows/cols harmlessly.
        kv_ps = ppool.tile([1 + C, 3 * D], F32, tag="ps_big", bufs=1)
        for j in range(3):
            for r in range(R):
                nc.tensor.matmul(kv_ps[:, j * D:(j + 1) * D],
                                 lhsT=pc_sb[:, r, :],
                                 rhs=wqkv_sb[:, r, j * D:(j + 1) * D],
                                 start=(r == 0), stop=(zb and r == R - 1))
            if not zb:
                nc.tensor.matmul(kv_ps[:, j * D:(j + 1) * D],
                                 lhsT=ones_b[0:1, 0:1 + C],
                                 rhs=bqkv_sb[0:1, j * D:(j + 1) * D],
                                 start=False, stop=True)
        q_row = singles.tile([1, D], BF16, tag="q_row")
        nc.scalar.copy(q_row, kv_ps[C:C + 1, 0:D])
        vh_sb = singles.tile([C, D], BF16, tag="vh_sb")
        nc.scalar.copy(vh_sb, kv_ps[0:C, 2 * D:3 * D])

        # scores^T [keys, heads] = sum_d (q (x) ones) * kh, per head
        qb_ps = ppool.tile([C, D], F32, tag="ps")
        nc.tensor.matmul(qb_ps, lhsT=ones_b[0:1, 0:C], rhs=q_row,
                         start=True, stop=True)
        qb_sb = singles.tile([C, D], F32, tag="qb_sb")
        nc.scalar.copy(qb_sb, qb_ps)
        s1 = singles.tile([C, D], F32, tag="s1")
        nc.vector.tensor_mul(s1, qb_sb, kv_ps[0:C, D:2 * D])
        sc_t = singles.tile([C, NH], F32, tag="sc_t")
        nc.vector.reduce_sum(sc_t, s1[:, :].rearrange("p (h d) -> p h d", h=NH),
                             axis=mybir.AxisListType.X)

        # transpose -> [heads, keys]; softmax over keys (scores are O(0.1):
        # exp without max-subtraction is safe at this weight scale)
        st_ps = ppool.tile([NH, C], F32, tag="ps")
        nc.tensor.transpose(st_ps, sc_t, ident[0:C, 0:C])
        wrow = singles.tile([NH, C], F32, tag="wrow")
        ssum = singles.tile([NH, 1], F32, tag="ssum")
        nc.scalar.activation(wrow, st_ps, AF.Exp, bias=0.0, scale=DH ** -0.5,
                             accum_out=ssum)
        rs = singles.tile([NH, 1], F32, tag="rs")
        nc.vector.reciprocal(rs, ssum)
        nc.vector.tensor_scalar_mul(wrow, wrow, rs)

        # w^T [keys, heads] for the per-head attention matmuls
        wT_ps = ppool.tile([C, NH], F32, tag="ps")
        nc.tensor.transpose(wT_ps, wrow, ident[0:NH, 0:NH])
        wT_sb = singles.tile([C, NH], BF16, tag="wT_sb")
        nc.scalar.copy(wT_sb, wT_ps)

        # attn1 (pre-out-proj) directly in column form [P, R]:
        # per head: [DH, 1] = vh_head^T . w_head
        a1c_ps = ppool.tile([P, R], F32, tag="ps")
        for hh in range(NH):
            po = DH * (hh % 2)
            nc.tensor.matmul(a1c_ps[po:po + DH, hh // 2:hh // 2 + 1],
                             lhsT=vh_sb[:, hh * DH:(hh + 1) * DH],
                             rhs=wT_sb[:, hh:hh + 1], start=True, stop=True)
        a1c_sb = singles.tile([P, R], BF16, tag="a1c_sb")
        nc.scalar.copy(a1c_sb, a1c_ps)

        # t = attn1 @ W_out1.T (+ b_out1) + pt    [1, D]
        t_ps = ppool.tile([1, D], F32, tag="ps")
        vec_mat(t_ps, a1c_sb, w_sb["w1oT"],
                (rows["pt"],) if zb else (rows["b1o"], rows["pt"]))

        # upd = LN1(t) (* ln1g + ln1b)
        st6 = singles.tile([1, 6], F32, tag="st6")
        nc.vector.bn_stats(st6, t_ps)
        mv1 = singles.tile([1, 2], F32, tag="mv1")
        nc.vector.bn_aggr(mv1, st6)
        sg1 = singles.tile([1, 1], F32, tag="sg1")
        nc.scalar.activation(sg1, mv1[0:1, 1:2], AF.Sqrt, bias=eps_col[0:1, :])
        rstd1 = singles.tile([1, 1], F32, tag="rstd1")
        nc.vector.reciprocal(rstd1, sg1)
        u_row = singles.tile([1, D], BF16, tag="u_row")
        nc.vector.tensor_scalar(u_row, t_ps, scalar1=mv1[0:1, 0:1],
                                scalar2=rstd1, op0=OP.subtract, op1=OP.mult)
        if not ln1_triv:
            nc.vector.tensor_mul(u_row, u_row, rows["ln1g"])
            nc.vector.tensor_add(u_row, u_row, rows["ln1b"])

        # v = upd @ Wv2.T (+ bv2);  attn2 = v @ W_out2.T (+ b_out2)
        uc_sb = row_to_col(u_row, "uc_sb")
        v_ps = ppool.tile([1, D], F32, tag="ps")
        vec_mat(v_ps, uc_sb, w_sb["wv2T"], () if zb else (rows["bv2"],))
        v_row = singles.tile([1, D], BF16, tag="v_row")
        nc.scalar.copy(v_row, v_ps)
        vc_sb = row_to_col(v_row, "vc_sb")
        a2_ps = ppool.tile([1, D], F32, tag="ps")
        vec_mat(a2_ps, vc_sb, w_sb["w2oT"], () if zb else (rows["b2o"],))
        a2_row = singles.tile([1, D], F32, tag="a2_row")
        nc.scalar.copy(a2_row, a2_ps)

        # ---------- main loop: out = LN2(attn2 + H) ----------
        # a_b is the attn2 row broadcast to all 128 partitions. Blocks of
        # 128*kb consecutive rows, p-major (partition p holds rows
        # [s+p*kb, s+(p+1)*kb)): one input dma_start per block with 16 KB
        # contiguous descriptors on the SP queue, one output dma_start per
        # block on the ACT queue. DVE/GpSimd split the broadcast adds, DVE
        # does stats, ACT normalizes in place. The loop is software-
        # pipelined one block deep so DVE keeps working while GpSimd's
        # slower adds finish.
        ab_ps = ppool.tile([P, D], F32, tag="ps")
        nc.tensor.matmul(ab_ps, lhsT=ones_f, rhs=a2_row, start=True, stop=True)
        a_b = singles.tile([P, D], F32, tag="a_b")
        nc.vector.tensor_copy(a_b, ab_ps)
        if general:
            def bcast_row(row_sb, tag):
                bps = ppool.tile([P, D], F32, tag="ps")
                nc.tensor.matmul(bps, lhsT=ones_f, rhs=row_sb,
                                 start=True, stop=True)
                bsb = singles.tile([P, D], F32, tag=tag)
                nc.vector.tensor_copy(bsb, bps)
                return bsb

            g2b = bcast_row(frows["ln2g"], "g2b")
            b2b = bcast_row(frows["ln2b"], "b2b")

        # separate broadcast copies per adder engine: DVE and GpSimd streaming
        # the SAME a_b tile concurrently showed heavy SBUF-conflict inflation
        # (632ns adds ballooning to ~1.7us)
        a_bg = singles.tile([P, D], F32, tag="a_bg")
        nc.vector.tensor_copy(a_bg, a_b)

        # (start_row, partitions, rows-per-partition) per block
        blocks = []
        s = 0
        while s + P * kb <= nsh:
            blocks.append((s, P, kb))
            s += P * kb
        rem = nsh - s
        if rem:
            # tail: pick rows-per-partition so partitions stay <= P
            kt = (rem + P - 1) // P
            while rem % kt:
                kt += 1
            blocks.append((s, rem // kt, kt))

        def front(blk):
            s, rp, kbb = blk
            x_t = xpool.tile([rp, kbb, D], F32, tag="x")
            nc.sync.dma_start(
                x_t, h[s:s + rp * kbb, :].rearrange("(p a) d -> p a d", p=rp))
            # DVE takes the first slices, GpSimd the rest (GpSimd's adds are
            # ~2x slower; DVE also carries the stats)
            ndve = max(1, (kbb * 3) // 8)
            for a in range(kbb):
                if a < ndve:
                    nc.vector.tensor_add(x_t[:, a, :], x_t[:, a, :],
                                         a_b[:rp, :])
                else:
                    nc.gpsimd.tensor_add(x_t[:, a, :], x_t[:, a, :],
                                         a_bg[:rp, :])
            return (blk, x_t)

        def back(state):
            (s, rp, kbb), x_t = state
            st = spool.tile([rp, kbb, 6], F32, tag="st")
            mv = spool.tile([rp, kbb, 2], F32, tag="mv")
            sg = spool.tile([rp, kbb], F32, tag="sg")
            nmr = spool.tile([rp, kbb], F32, tag="nmr")
            for a in range(kbb):
                nc.vector.bn_stats(st[:, a, :], x_t[:, a, :])
                nc.vector.bn_aggr(mv[:, a, :], st[:, a, :])
            nc.scalar.activation(sg, mv[:, :, 1], AF.Sqrt, bias=eps_col[:rp, :])
            nc.vector.reciprocal(sg, sg)
            nc.vector.tensor_mul(nmr, mv[:, :, 0], sg)
            nc.vector.tensor_scalar_mul(nmr, nmr, -1.0)
            for a in range(kbb):
                nc.scalar.activation(x_t[:, a, :], x_t[:, a, :], AF.Identity,
                                     bias=nmr[:, a:a + 1], scale=sg[:, a:a + 1])
                if general:
                    nc.vector.tensor_mul(x_t[:, a, :], x_t[:, a, :], g2b[:rp, :])
                    nc.vector.tensor_add(x_t[:, a, :], x_t[:, a, :], b2b[:rp, :])
            nc.scalar.dma_start(
                o_dram[s:s + rp * kbb, :].rearrange("(p a) d -> p a d", p=rp),
                x_t)

        prev = None
        for blk in blocks:
            cur = front(blk)
            if prev is not None:
                back(prev)
            prev = cur
        back(prev)

    nc.compile()
    return nc


def _get_nc(nsh, kb, general, zb, ln1_triv):
    key = (nsh, kb, general, zb, ln1_triv)
    if key not in _cache:
        _cache[key] = _build(nsh, kb, general, zb, ln1_triv)
    return _cache[key]


def _prep_in_maps(inputs: dict):
    f = lambda x: np.ascontiguousarray(np.asarray(x), dtype=np.float32)
    bf = lambda x: np.ascontiguousarray(np.asarray(x, dtype=np.float32),
                                        dtype=ml_dtypes.bfloat16)
    H = f(inputs["H"])
    assert H.shape == (N, D), H.shape
    cid = np.asarray(inputs["context_ids"]).astype(np.int64)
    W_in1 = f(inputs["W_in1"])
    b_in1 = f(inputs["b_in1"])
    W_in2 = f(inputs["W_in2"])
    b_in2 = f(inputs["b_in2"])
    ln1g, ln1b = f(inputs["ln1_g"]), f(inputs["ln1_b"])
    ln2g, ln2b = f(inputs["ln2_g"]), f(inputs["ln2_b"])
    biases = (b_in1, f(inputs["b_out1"]), b_in2[2 * D:], f(inputs["b_out2"]))
    zb = all(np.all(b == 0.0) for b in biases)
    ln1_triv = bool(np.all(ln1g == 1.0) and np.all(ln1b == 0.0))
    general = not (np.all(ln2g == 1.0) and np.all(ln2b == 0.0))
    pt32 = f(inputs["path_token"])
    common = {
        "pcT": bf(np.concatenate([H[cid].T, pt32[:, None]], axis=1)),
        "pt": bf(pt32),
        "wqkvT": bf(W_in1.T),
        "bqkv": bf(b_in1),
        "w1oT": bf(f(inputs["W_out1"]).T),
        "wv2T": bf(W_in2[2 * D:3 * D].T),
        "w2oT": bf(f(inputs["W_out2"]).T),
        "b1o": bf(inputs["b_out1"]),
        "bv2": bf(b_in2[2 * D:3 * D]),
        "b2o": bf(inputs["b_out2"]),
        "ln1g": bf(ln1g),
        "ln1b": bf(ln1b),
        "ln2g": ln2g,
        "ln2b": ln2b,
    }
    shards = np.split(H, N_CORES, axis=0)
    in_maps = [dict(common, h=shards[i]) for i in range(N_CORES)]
    return in_maps, (general, zb, ln1_triv)


def _run(inputs: dict, trace: bool = False):
    in_maps, (general, zb, ln1_triv) = _prep_in_maps(inputs)
    nc = _get_nc(NSH, KB, general, zb, ln1_triv)
    res = run_bass_kernel_spmd(nc, in_maps, core_ids=list(range(N_CORES)),
                               trace=trace)
    out = np.concatenate([res.results[i]["out"] for i in range(N_CORES)], axis=0)
    return out, res


def kernel(**inputs) -> np.ndarray:
    out, _ = _run(inputs)
    return out
